# revision 1
# baseline (speedup 1.0000x reference)
"""DCN CrossLayer kernel for Trainium2 (8 NeuronCores, batch-sharded).

Math: the reference loop
    cross = x
    for i in range(L):
        s_i   = sum(cross, axis=1)                  # (B, 1)
        cross = s_i * x * W[i] + b[i] + cross
collapses to
    out[b, k] = x[b, k] * (1 + sum_i s_i[b] * W[i, k]) + Bsum[k]
with
    u_i[b]  = sum_k x[b, k] * W[i, k]
    s_0[b]  = sum_k x[b, k]
    s_{i+1} = s_i * (1 + u_i) + beta_i,   beta_i = sum_k b[i, k]
    Bsum[k] = sum_i b[i, k]

Layout strategy (v2): the host uploads x PRE-TRANSPOSED and in fp16
(x^T: [D, rows], k on partitions).  This halves the input HBM bytes
(8 MiB -> 4 MiB per core) and removes all 128 big PE transposes the
natural layout needs: the k-contraction for [s_0, u_i] is a direct
PE matmul U = A^T @ x^T accumulated over 16 k-chunks, and the final
product is computed transposed, out^T = x^T * T^T with
T^T[k, b] = 1 + sum_i W[i, k] s'_i[b]  (one [5]-deep matmul per
[128 k x 512 b] chunk).  The host transposes the fp16 result back.

Per-core schedule: the 1024-row b-range splits into two 512-wide
halves.  Half 0's 16 k-chunk loads stream first, so its U/recursion/
T-multiply/store pipeline runs while half 1 loads; the final load
batches shrink to single chunks so U catches up during the stream.
The elementwise multiply (the only pass that must touch every output
element on a compute engine) is split DVE (reads T from PSUM
directly) / ACT-copy+GPSIMD / ACT-copy+DVE-fp16 so no single engine
paces the tail.  CoreSim DMA floor: 8 MiB @ 360 GB/s = 23.3 us.

Precision: fp16 x quantization ~2.4e-4, fp16 store ~2.4e-4, s'
chain ~3e-4 -> total rel err ~5e-4 (gate is 2e-2).
"""

import sys

sys.path.insert(0, "/opt/trn_rl_repo")

import numpy as np

import concourse.bacc as bacc
import concourse.tile as tile
from concourse import mybir
from concourse.bass_utils import run_bass_kernel_spmd
from concourse.masks import make_identity

N_CORES = 8
B, D, L = 8192, 2048, 4
RB = B // N_CORES            # 1024 batch rows per core
P = 128                      # partitions
KC = D // P                  # 16 k-chunks of 128
NH = 2                       # b halves per core
HW = RB // NH                # 512 b columns per half
NSUB = HW // P               # 4 recursion subtiles per half

F32 = mybir.dt.float32
F16 = mybir.dt.float16
ADD = mybir.AluOpType.add
MULT = mybir.AluOpType.mult

# Chunks per load DMA within a half (sums to KC).  Tail batches shrink to
# single chunks so the U accumulation catches up while the stream finishes.
LOAD_BATCHES = (4, 4, 4, 2, 1, 1)
# Chunks per store DMA within a half (sums to KC).
STORE_GROUPS = (4, 4, 4, 2, 2)
# Multiply path per chunk, per half: 'd' = DVE reads T from PSUM directly,
# 'g' = ACT copies T to fp16 SBUF + GPSIMD multiply, 'a' = ACT copy + DVE
# fp16 multiply (2x mode).  Interleaved so all engines start immediately.
PATHS = {
    0: "dgdgdgdgdgdgdgdd",
    1: "dgdgdgdgdgdgdgdd",
}


def build_program(betas):
    """Build the per-core Bass program (same program on all 8 cores)."""
    nc = bacc.Bacc("TRN2", target_bir_lowering=False)

    xt_d = nc.dram_tensor("xt", [D, RB], F16, kind="ExternalInput")
    a_d = nc.dram_tensor("acoef", [P, KC * L], F16, kind="ExternalInput")
    wv_d = nc.dram_tensor("wv", [L + 1, D], F16, kind="ExternalInput")
    out_d = nc.dram_tensor("out", [D, RB], F16, kind="ExternalOutput")

    xt_t = xt_d.rearrange("(c p) b -> p c b", p=P)
    out_t = out_d.rearrange("(c p) b -> p c b", p=P)

    with tile.TileContext(nc) as tc:
        with (
            tc.tile_pool(name="consts", bufs=1) as consts,
            tc.tile_pool(name="xp", bufs=1) as xp,
            tc.tile_pool(name="op", bufs=1) as op,
            tc.tile_pool(name="smalls", bufs=8) as smalls,
            tc.tile_pool(name="tsbp", bufs=4) as tsbp,
            tc.tile_pool(name="u_ps", bufs=1, space="PSUM") as u_ps,
            tc.tile_pool(name="st_ps", bufs=1, space="PSUM") as st_ps,
            tc.tile_pool(name="t_ps", bufs=4, space="PSUM") as t_ps,
        ):
            # x loads lead on the SP ring; tiny consts ride SWDGE (no HWDGE
            # slot) so they only displace ~150ns of the x stream.
            xall = xp.tile([P, KC, RB], F16, tag="x")
            load_ranges = []
            c0 = 0
            for nb in LOAD_BATCHES:
                load_ranges.append((c0, c0 + nb))
                c0 += nb
            for h in range(NH):
                hs = slice(h * HW, (h + 1) * HW)
                for lo, hi in load_ranges:
                    nc.sync.dma_start(
                        out=xall[:, lo:hi, hs], in_=xt_t[:, lo:hi, hs]
                    )
                if h == 0:
                    a_sb = consts.tile([P, KC * L], F16)
                    nc.gpsimd.dma_start(out=a_sb, in_=a_d[:])
                    wv_sb = consts.tile([L + 1, D], F16)
                    nc.gpsimd.dma_start(out=wv_sb, in_=wv_d[:])
                    ident = consts.tile([P, P], F32)
                    make_identity(nc, ident)

            oall = op.tile([P, KC, RB], F16, tag="o")
            # Full-bank U tiles: rows 0..3 hold the U accumulation; after the
            # ACT evacuation the same bank is reused as the transpose target
            # for the natural-layout u (saves a PSUM bank, and the WAR dep is
            # exactly the evacuation).
            u_tiles = [
                u_ps.tile([P, HW], F32, tag=f"u{h}", name=f"u{h}")
                for h in range(NH)
            ]
            st_tiles = [None, None]

            def u_mms(h, lo, hi):
                """U^T accumulation matmuls for chunks [lo, hi) of half h."""
                hs = slice(h * HW, (h + 1) * HW)
                for c in range(lo, hi):
                    nc.tensor.matmul(
                        u_tiles[h][:L, :],
                        a_sb[:, c * L : (c + 1) * L],
                        xall[:, c, hs],
                        start=(c == 0),
                        stop=(c == KC - 1),
                    )

            def recursion(h):
                """U -> S' for half h.

                The [4, 512] U rows live on partitions 1..3, which compute
                engines cannot address individually (mod-32 base rule), so
                transpose to natural [128, sub, i] layout, run the chain on
                GPSIMD (keeps DVE free for multiplies), transpose back.
                """
                u_sb = smalls.tile([L, HW], F32, tag=f"usb{h}")
                nc.scalar.copy(u_sb, u_tiles[h][:L, :])
                un_ps = u_tiles[h]
                for s in range(NSUB):
                    nc.tensor.transpose(
                        un_ps[:, s * L : (s + 1) * L],
                        u_sb[:, s * P : (s + 1) * P],
                        ident[:L, :L],
                    )
                un_v = un_ps[:, : NSUB * L].rearrange("p (s l) -> p s l", s=NSUB)
                sn = smalls.tile([P, NSUB, L + 1], F32, tag=f"sn{h}")
                nc.gpsimd.memset(sn[:, :, L], 1.0)
                nc.scalar.copy(sn[:, :, 0], un_v[:, :, 0])
                if all(bt == 0.0 for bt in betas):
                    # ACT evacuates 1+u_i in one fused op; the chain is then
                    # three plain multiplies, which Pool supports (keeps DVE
                    # free; TensorScalarPtr is not a Pool instruction).
                    un1 = smalls.tile([P, NSUB, L - 1], F32, tag=f"un{h}")
                    nc.scalar.add(un1, un_v[:, :, 1:], 1.0)
                    for i in range(L - 1):
                        nc.gpsimd.tensor_mul(
                            sn[:, :, i + 1], sn[:, :, i], un1[:, :, i]
                        )
                else:
                    for i in range(L - 1):
                        nc.vector.scalar_tensor_tensor(
                            out=sn[:, :, i + 1],
                            in0=un_v[:, :, i + 1],
                            scalar=1.0,
                            in1=sn[:, :, i],
                            op0=ADD,
                            op1=MULT,
                        )
                        nc.vector.tensor_scalar_add(
                            sn[:, :, i + 1], sn[:, :, i + 1], float(betas[i])
                        )
                stp = st_ps.tile([L + 1, HW], F32, tag=f"stp{h}")
                for s in range(NSUB):
                    nc.tensor.transpose(
                        stp[:, s * P : (s + 1) * P], sn[:, s, :], ident
                    )
                st = smalls.tile([L + 1, HW], F16, tag=f"st{h}")
                nc.scalar.copy(st, stp)
                st_tiles[h] = st

            def v_mult(h, lo, hi):
                """T^T matmul + elementwise multiply for chunks [lo, hi)."""
                hs = slice(h * HW, (h + 1) * HW)
                for c in range(lo, hi):
                    tp = t_ps.tile([P, 512], F32, tag="t")
                    nc.tensor.matmul(
                        tp,
                        wv_sb[:, c * P : (c + 1) * P],
                        st_tiles[h],
                        start=True,
                        stop=True,
                    )
                    path = PATHS[h][c]
                    if path == "d":
                        nc.vector.tensor_mul(oall[:, c, hs], xall[:, c, hs], tp)
                    else:
                        tsb = tsbp.tile([P, 512], F16, tag="tsb")
                        nc.scalar.copy(tsb, tp)
                        eng = nc.gpsimd if path == "g" else nc.vector
                        eng.tensor_mul(oall[:, c, hs], xall[:, c, hs], tsb)

            def stores(h, groups):
                hs = slice(h * HW, (h + 1) * HW)
                c0 = 0
                for g in groups:
                    nc.sync.dma_start(
                        out=out_t[:, c0 : c0 + g, hs],
                        in_=oall[:, c0 : c0 + g, hs],
                    )
                    c0 += g

            # Emission order sets scheduler PRIORITY (the Tile list scheduler
            # pops the lowest-priority READY instruction per engine).  All U
            # matmuls and both recursions lead, so they jump engine queues
            # the moment their data lands; the V/multiply/store pipelines
            # backfill every idle slot without ever delaying the U chain.
            u_mms(0, 0, KC)
            u_mms(1, 0, KC)
            recursion(0)
            recursion(1)
            v_mult(0, 0, KC)
            stores(0, STORE_GROUPS)
            v_mult(1, 0, KC)
            stores(1, STORE_GROUPS)

    nc.finalize()
    return nc


_CACHE = {}


def _get_program(betas):
    key = tuple(float(b) for b in betas)
    if key not in _CACHE:
        _CACHE[key] = build_program(key)
    return _CACHE[key]


def make_in_maps(x, W, b):
    """Shard x (fp16, transposed) across cores; replicate coefficients."""
    x = np.asarray(x, dtype=np.float32)
    W = np.asarray(W, dtype=np.float32)
    assert x.shape == (B, D) and W.shape == (L, D)

    x16 = x.astype(np.float16)
    # A = [ones, W0, W1, W2] as [P, KC*L]: a[p, c*L+i] = A[c*128+p, i]
    a_mat = np.concatenate([np.ones((D, 1), np.float32), W[: L - 1].T], axis=1)
    a_host = np.ascontiguousarray(
        a_mat.reshape(KC, P, L).transpose(1, 0, 2).reshape(P, KC * L)
    ).astype(np.float16)
    # W'' = [W; ones] as [L+1, D]
    wv_host = np.concatenate([W, np.ones((1, D), np.float32)], axis=0).astype(
        np.float16
    )
    return [
        {
            "xt": np.ascontiguousarray(x16[i * RB : (i + 1) * RB].T),
            "acoef": a_host,
            "wv": wv_host,
        }
        for i in range(N_CORES)
    ]


def kernel(**inputs) -> np.ndarray:
    x = np.asarray(inputs["x"], dtype=np.float32)
    W = np.asarray(inputs["W"], dtype=np.float32)
    b = np.asarray(inputs["b"], dtype=np.float32)

    betas = b.sum(axis=1, dtype=np.float64).astype(np.float32)
    nc = _get_program(betas)
    in_maps = make_in_maps(x, W, b)
    res = run_bass_kernel_spmd(nc, in_maps, list(range(N_CORES)))
    out = np.concatenate(
        [res.results[i]["out"].T for i in range(N_CORES)], axis=0
    ).astype(np.float32)

    bsum = b.sum(axis=0, dtype=np.float64).astype(np.float32)
    if np.any(bsum != 0.0):
        out = out + bsum[None, :]
    return out

